# revision 18
# baseline (speedup 1.0000x reference)
"""Trainium2 Bass kernel for nn_Attention_9594956939856.

Single-head spatial self-attention over 64x64 feature maps:
    q = Wq@x, k = Wk@x, v = Wv@x  (1x1 convs over channels)
    out = gamma * softmax(q^T k) @ v + x

Sharding: data-parallel over batch — 8 samples onto 8 NeuronCores, each core
computes one full sample (C=256, N=4096 tokens, dk=32). No collectives.

Per-core layout strategy (matmuls on TensorE compute out = lhsT.T @ rhs):
  - scores are computed directly TRANSPOSED: s'[j,i] = sum_d k[d,j] q[d,i]
    with k j-tiles stationary, so the huge attention matrix never needs a
    transpose. q/k are replicated 4x along partitions (via host-replicated
    W^T) so the K=32 contraction can later use 4x row-tiled matmuls.
  - softmax denominator: ones(128,128) stationary sums exp(s') over
    partitions (j), accumulated across j-tiles in PSUM; M=128 broadcasts the
    sum to every output partition for free.
  - v is produced directly in transposed layout vT[n,e] by the projection
    (lhsT = x chunks, rhs = Wv^T) — exactly the stationary layout the
    attention-weighted sum needs.
  - exp on ScalarE in (128,1024) chunks (bf16 out), fp32 PSUM accumulation.
    Scores are in [-5,5] for this input distribution, so softmax without
    max-subtraction is numerically safe.
"""

import ml_dtypes
import numpy as np

import concourse.bass as bass
import concourse.mybir as mybir
from concourse.tile import TileContext
from concourse.bass_utils import run_bass_kernel_spmd

B, C, H, W = 8, 256, 64, 64
N = H * W          # 4096 tokens
DK = C // 8        # 32
P = 128
F32 = mybir.dt.float32
F32R = mybir.dt.float32r  # fp32 storage, single-pass (4x faster) PE streaming
BF16 = mybir.dt.bfloat16
AF = mybir.ActivationFunctionType
ALU = mybir.AluOpType

NJT = N // P       # 32 j-tiles
ICH = 1024         # i-chunk width for the scores'/exp stage
NICH = N // ICH    # 4
HCH = 512          # accumulation sub-chunk (one PSUM bank)


# ---------------------------------------------------------------------------
# Workaround: the walrus build in this container allows only ONE sync wait
# per instruction ("Too many sync wait commands"), but Tile's wait
# assignment attaches up to 2 (and the tail drain more). Hoist all-but-one
# wait of any over-subscribed instruction onto dedicated same-engine nofuse
# nops inserted immediately before it in the ordered stream.
_PATCHED = False


def _apply_tile_patch():
    global _PATCHED
    if _PATCHED:
        return
    from concourse.tile import TileContext as TC
    from concourse.vector_clock import ScopedClock, VectorClock

    def _drain_and_barrier_split(self, tick_clock, wait_clock):
        gc = tick_clock.global_clock
        n = len(gc)
        for i in range(n):
            if gc[i] > 0:
                vec = [0] * n
                vec[i] = gc[i]
                ins = self.nc.sync.nop(nofuse=True, hint="tail_drain_wait")
                wait_clock.add_sem_waits(
                    ins.ins, ScopedClock({None: VectorClock(vec)})
                )
        self.nc.sync.drain()
        self.nc.all_engine_barrier()
        assert self.sems is not None
        popped = self.nc._tile_sem_poison_stack.pop()
        assert popped is self._sem_poison
        self.nc.clear_and_free_semaphores(list(self.sems.allocated().values()))
        self.nc.all_engine_barrier()

    TC._drain_and_barrier = _drain_and_barrier_split

    orig_lower = TC._lower_ordered_insts
    counter = [0]

    def _lower_split_waits(self, ordered):
        for bb_name, insts in ordered.items():
            new = []
            changed = False
            for inst in insts:
                si = inst.sync_info
                if si is not None and len(si.on_wait) > 1:
                    changed = True
                    waits = list(si.on_wait)
                    for w in waits[:-1]:
                        counter[0] += 1
                        new.append(
                            mybir.InstNoOp(
                                name=f"splitw-{counter[0]}",
                                sync_info=mybir.SyncInfo(
                                    on_wait=[w], on_update=[]
                                ),
                                bass_nofuse=True,
                                engine=inst.engine,
                            )
                        )
                    inst.sync_info = mybir.SyncInfo(
                        on_wait=[waits[-1]], on_update=list(si.on_update)
                    )
                new.append(inst)
            if changed:
                insts[:] = new
        return orig_lower(self, ordered)

    TC._lower_ordered_insts = _lower_split_waits
    _PATCHED = True


def _emit_body(nc, tc, pools, ext):
    """Emit one full attention computation (one sample)."""
    consts, big, epool, fin, ps_s_pool, ps_acc_pool = pools
    x_e, wqt_e, wkt_e, wvt_e, bq_e, bk_e, bv_e, gam_e, y_e = ext

    # ---- constants / weights ---------------------------------------------
    wqt_a = consts.tile([P, P], BF16, tag="wqt_a")
    wqt_b = consts.tile([P, P], BF16, tag="wqt_b")
    wkt_a = consts.tile([P, P], BF16, tag="wkt_a")
    wkt_b = consts.tile([P, P], BF16, tag="wkt_b")
    wvt_a = consts.tile([P, C], BF16, tag="wvt_a")
    wvt_b = consts.tile([P, C], BF16, tag="wvt_b")
    bq_t = consts.tile([P, 1], F32, tag="bq_t")
    bk_t = consts.tile([P, 1], F32, tag="bk_t")
    bv_t = consts.tile([P, C], F32, tag="bv_t")
    gam_t = consts.tile([P, 1], F32, tag="gam_t")
    ones = consts.tile([P, P], BF16, tag="ones")
    ones_f = consts.tile([P, P], F32, tag="ones_f")

    nc.sync.dma_start(out=wqt_a[:], in_=wqt_e[0:P, :])
    nc.sync.dma_start(out=wqt_b[:], in_=wqt_e[P : 2 * P, :])
    nc.sync.dma_start(out=wkt_a[:], in_=wkt_e[0:P, :])
    nc.sync.dma_start(out=wkt_b[:], in_=wkt_e[P : 2 * P, :])
    nc.sync.dma_start(out=wvt_a[:], in_=wvt_e[0:P, :])
    nc.sync.dma_start(out=wvt_b[:], in_=wvt_e[P : 2 * P, :])
    nc.sync.dma_start(out=bq_t[:], in_=bq_e[:])
    nc.sync.dma_start(out=bk_t[:], in_=bk_e[:])
    nc.sync.dma_start(out=bv_t[:], in_=bv_e[:])
    nc.sync.dma_start(out=gam_t[:], in_=gam_e[:])
    nc.vector.memset(ones[:], 1.0)
    nc.vector.memset(ones_f[:], 1.0)

    xf0 = big.tile([P, N], F32, tag="xf0")
    xf1 = big.tile([P, N], F32, tag="xf1")
    xb0 = big.tile([P, N], BF16, tag="xb0")
    xb1 = big.tile([P, N], BF16, tag="xb1")
    q_rep = big.tile([P, N], BF16, tag="q_rep")
    k_rep = big.tile([P, N], BF16, tag="k_rep")
    vt = big.tile([P, NJT * C], BF16, tag="vt")

    # ---- chunked x load + bf16 cast + q/k projections --------------------
    for nch in range(N // HCH):
        sl = slice(nch * HCH, (nch + 1) * HCH)
        nc.sync.dma_start(out=xf0[:, sl], in_=x_e[0:P, sl])
        nc.sync.dma_start(out=xf1[:, sl], in_=x_e[P : 2 * P, sl])
        nc.vector.tensor_copy(xb0[:, sl], xf0[:, sl])
        nc.vector.tensor_copy(xb1[:, sl], xf1[:, sl])
        pk = ps_acc_pool.tile([P, HCH], F32, tag="po", bufs=2)
        nc.tensor.matmul(pk[:], wkt_a[:], xb0[:, sl], start=True, stop=False)
        nc.tensor.matmul(pk[:], wkt_b[:], xb1[:, sl], start=False, stop=True)
        nc.vector.tensor_scalar_add(k_rep[:, sl], pk[:], bk_t[:])
    for nch in range(N // HCH):
        sl = slice(nch * HCH, (nch + 1) * HCH)
        pq = ps_acc_pool.tile([P, HCH], F32, tag="po", bufs=2)
        nc.tensor.matmul(pq[:], wqt_a[:], xb0[:, sl], start=True, stop=False)
        nc.tensor.matmul(pq[:], wqt_b[:], xb1[:, sl], start=False, stop=True)
        nc.vector.tensor_scalar_add(q_rep[:, sl], pq[:], bq_t[:])

    # ---- skewed pipeline over i-chunks of 512 ----------------------------
    # Iteration pich: produce exp-scores e(pich+1) on ACT while PE chews the
    # accumulation matmuls of e(pich); pich == -1 is the prologue whose PE
    # filler is the vT projection.
    NCH = N // HCH  # 8
    NJG = NJT // 4  # 8 j-groups of 4 j-tiles

    def s_group(jg, ich, nxt_e):
        """4x row-tiled scores' + one exp for j-tiles 4jg..4jg+3, i-chunk ich."""
        isl = slice(ich * HCH, (ich + 1) * HCH)
        ps_big = ps_s_pool.tile([P, 4 * HCH], F32, tag="ps_s")
        for g in range(4):
            jt = 4 * jg + g
            nc.tensor.matmul(
                ps_big[:, g * HCH : (g + 1) * HCH],
                k_rep[32 * g : 32 * (g + 1), jt * P : (jt + 1) * P],
                q_rep[32 * g : 32 * (g + 1), isl],
                start=True, stop=True,
                tile_position=(32 * g, 0),
            )
        e_big = epool.tile([P, 4 * HCH], BF16, tag="e")
        nc.scalar.activation(e_big[:], ps_big[:], AF.Exp)
        nxt_e.append(e_big)

    cur_e = []
    for pich in range(-1, NCH):
        nxt_e = []
        produce = pich + 1 < NCH
        if pich >= 0:
            po0 = ps_acc_pool.tile([P, HCH], F32, tag="po", bufs=2)
            po1 = ps_acc_pool.tile([P, HCH], F32, tag="po", bufs=2)
            pd = ps_acc_pool.tile([P, HCH], F32, tag="pd", bufs=2)
        for jg in range(NJG):
            if produce:
                s_group(jg, pich + 1, nxt_e)
            if pich == -1:
                # prologue PE filler: vT projection for 4 j-tiles
                for jt in range(4 * jg, 4 * jg + 4):
                    nsl = slice(jt * P, (jt + 1) * P)
                    pv = ps_acc_pool.tile([P, C], F32, tag="po", bufs=2)
                    nc.tensor.matmul(
                        pv[:], xb0[:, nsl], wvt_a[:], start=True, stop=False
                    )
                    nc.tensor.matmul(
                        pv[:], xb1[:, nsl], wvt_b[:], start=False, stop=True
                    )
                    nc.vector.tensor_tensor(
                        vt[:, jt * C : (jt + 1) * C], pv[:], bv_t[:], op=ALU.add
                    )
            else:
                for jt in range(4 * jg, 4 * jg + 4):
                    e_sl = cur_e[jt // 4][:, (jt % 4) * HCH : (jt % 4 + 1) * HCH]
                    st = jt == 0
                    sp = jt == NJT - 1
                    nc.tensor.matmul(
                        po0[:], vt[:, jt * C : jt * C + P], e_sl,
                        start=st, stop=sp,
                    )
                    nc.tensor.matmul(
                        po1[:], vt[:, jt * C + P : (jt + 1) * C], e_sl,
                        start=st, stop=sp,
                    )
                    g = jt % 4
                    nc.tensor.matmul(
                        pd[32 * g : 32 * (g + 1), :], ones[:, 0:32], e_sl,
                        start=(jt < 4), stop=(jt >= NJT - 4),
                        tile_position=(0, 32 * g),
                    )
        if pich >= 0:
            # quadrant-sum (32x overcount folded into gamma), finalize
            isl = slice(pich * HCH, (pich + 1) * HCH)
            d_sb = fin.tile([P, HCH], F32, tag="d_sb")
            nc.vector.tensor_copy(d_sb[:], pd[:])
            nc.tensor.matmul(pd[:], ones_f[:], d_sb[:], start=True, stop=True)
            dr = fin.tile([P, HCH], F32, tag="dr")
            nc.vector.reciprocal(dr[:], pd[:])
            nc.vector.tensor_scalar_mul(dr[:], dr[:], gam_t[:])
            t0 = fin.tile([P, HCH], F32, tag="t0")
            nc.vector.tensor_tensor(t0[:], po0[:], dr[:], op=ALU.mult)
            nc.vector.tensor_tensor(t0[:], t0[:], xf0[:, isl], op=ALU.add)
            nc.sync.dma_start(out=y_e[0:P, isl], in_=t0[:])
            t1 = fin.tile([P, HCH], F32, tag="t1")
            nc.vector.tensor_tensor(t1[:], po1[:], dr[:], op=ALU.mult)
            nc.vector.tensor_tensor(t1[:], t1[:], xf1[:, isl], op=ALU.add)
            nc.sync.dma_start(out=y_e[P : 2 * P, isl], in_=t1[:])
        cur_e = nxt_e


def build_bass(loop_n: int | None = None) -> bass.Bass:
    """Build the kernel. loop_n wraps the body in a device-side For_i loop
    (with a tiny 'tick' sentinel output) for slope-based benchmarking."""
    _apply_tile_patch()
    nc = bass.Bass()

    x_e = nc.declare_dram_parameter("x", [C, N], F32, isOutput=False)
    wqt_e = nc.declare_dram_parameter("wqt", [C, P], BF16, isOutput=False)
    wkt_e = nc.declare_dram_parameter("wkt", [C, P], BF16, isOutput=False)
    wvt_e = nc.declare_dram_parameter("wvt", [C, C], BF16, isOutput=False)
    bq_e = nc.declare_dram_parameter("bq_r", [P, 1], F32, isOutput=False)
    bk_e = nc.declare_dram_parameter("bk_r", [P, 1], F32, isOutput=False)
    bv_e = nc.declare_dram_parameter("bv_b", [P, C], F32, isOutput=False)
    gam_e = nc.declare_dram_parameter("gam_b", [P, 1], F32, isOutput=False)
    y_e = nc.declare_dram_parameter("y", [C, N], F32, isOutput=True)
    tick_e = None
    if loop_n is not None:
        tick_e = nc.declare_dram_parameter("tick", [1, 8], F32, isOutput=True)

    ext = (x_e, wqt_e, wkt_e, wvt_e, bq_e, bk_e, bv_e, gam_e, y_e)

    with (
        TileContext(nc) as tc,
        tc.tile_pool(name="consts", bufs=1) as consts,
        tc.tile_pool(name="big", bufs=1) as big,
        tc.tile_pool(name="epool", bufs=18) as epool,
        tc.tile_pool(name="fin", bufs=2) as fin,
        tc.tile_pool(name="ps_s", bufs=1, space="PSUM") as ps_s_pool,
        tc.tile_pool(name="ps_acc", bufs=3, space="PSUM") as ps_acc_pool,
    ):
        pools = (consts, big, epool, fin, ps_s_pool, ps_acc_pool)
        if loop_n is None:
            _emit_body(nc, tc, pools, ext)
        else:
            with tc.For_i(0, loop_n, 1):
                _emit_body(nc, tc, pools, ext)
            t = fin.tile([1, 8], F32, tag="tick")
            nc.vector.memset(t[:], 1.0)
            nc.sync.dma_start(out=tick_e[:], in_=t[:])

    return nc


_NC_CACHE = None


def _get_nc() -> bass.Bass:
    global _NC_CACHE
    if _NC_CACHE is None:
        _NC_CACHE = build_bass()
    return _NC_CACHE


def prep_core_inputs(x, Wq, bq, Wk, bk, Wv, bv, gamma):
    x = np.asarray(x, np.float32).reshape(B, C, N)
    wqt = np.ascontiguousarray(np.tile(np.asarray(Wq, np.float32).T, (1, 4))).astype(ml_dtypes.bfloat16)
    wkt = np.ascontiguousarray(np.tile(np.asarray(Wk, np.float32).T, (1, 4))).astype(ml_dtypes.bfloat16)
    wvt = np.ascontiguousarray(np.asarray(Wv, np.float32).T).astype(ml_dtypes.bfloat16)
    bq_r = np.ascontiguousarray(np.tile(np.asarray(bq, np.float32), 4)).reshape(P, 1)
    bk_r = np.ascontiguousarray(np.tile(np.asarray(bk, np.float32), 4)).reshape(P, 1)
    bv_b = np.ascontiguousarray(np.broadcast_to(np.asarray(bv, np.float32), (P, C)))
    # NOTE: quadrant strip-sum replicates each quadrant sum over 32 rows, so
    # the all-ones reduction yields 32x the true denominator; compensate here.
    gam_b = np.full((P, 1), 32.0 * float(np.asarray(gamma).reshape(-1)[0]), np.float32)
    shared = {
        "wqt": wqt, "wkt": wkt, "wvt": wvt,
        "bq_r": bq_r, "bk_r": bk_r, "bv_b": bv_b, "gam_b": gam_b,
    }
    return [{"x": np.ascontiguousarray(x[b]), **shared} for b in range(B)]


def kernel(**inputs) -> np.ndarray:
    nc = _get_nc()
    in_maps = prep_core_inputs(**inputs)
    res = run_bass_kernel_spmd(nc, in_maps, list(range(B)))
    y = np.stack([res.results[i]["y"] for i in range(B)])
    return np.ascontiguousarray(y.reshape(B, C, H, W).astype(np.float32))


# revision 21
# speedup vs baseline: 1.1663x; 1.1663x over previous
"""Trainium2 Bass kernel for nn_Attention_9594956939856.

Single-head spatial self-attention over 64x64 feature maps:
    q = Wq@x, k = Wk@x, v = Wv@x  (1x1 convs over channels)
    out = gamma * softmax(q^T k) @ v + x

Sharding: data-parallel over batch — 8 samples onto 8 NeuronCores, each core
computes one full sample (C=256, N=4096 tokens, dk=32). No collectives.

Per-core layout strategy (matmuls on TensorE compute out = lhsT.T @ rhs):
  - scores are computed directly TRANSPOSED: s'[j,i] = sum_d k[d,j] q[d,i]
    with k j-tiles stationary, so the huge attention matrix never needs a
    transpose. q/k are replicated 4x along partitions (via host-replicated
    W^T) so the K=32 contraction can later use 4x row-tiled matmuls.
  - softmax denominator: ones(128,128) stationary sums exp(s') over
    partitions (j), accumulated across j-tiles in PSUM; M=128 broadcasts the
    sum to every output partition for free.
  - v is produced directly in transposed layout vT[n,e] by the projection
    (lhsT = x chunks, rhs = Wv^T) — exactly the stationary layout the
    attention-weighted sum needs.
  - exp on ScalarE in (128,1024) chunks (bf16 out), fp32 PSUM accumulation.
    Scores are in [-5,5] for this input distribution, so softmax without
    max-subtraction is numerically safe.
"""

import ml_dtypes
import numpy as np

import concourse.bass as bass
import concourse.mybir as mybir
from concourse.tile import TileContext
from concourse.bass_utils import run_bass_kernel_spmd

B, C, H, W = 8, 256, 64, 64
N = H * W          # 4096 tokens
DK = C // 8        # 32
P = 128
F32 = mybir.dt.float32
F32R = mybir.dt.float32r  # fp32 storage, single-pass (4x faster) PE streaming
BF16 = mybir.dt.bfloat16
AF = mybir.ActivationFunctionType
ALU = mybir.AluOpType

NJT = N // P       # 32 j-tiles
ICH = 1024         # i-chunk width for the scores'/exp stage
NICH = N // ICH    # 4
HCH = 512          # accumulation sub-chunk (one PSUM bank)

# A/B flags (module-level so experiments can flip them before build)
VARIANT = {"s_tiled": True, "d_tiled": True}


# ---------------------------------------------------------------------------
# Workaround: the walrus build in this container allows only ONE sync wait
# per instruction ("Too many sync wait commands"), but Tile's wait
# assignment attaches up to 2 (and the tail drain more). Hoist all-but-one
# wait of any over-subscribed instruction onto dedicated same-engine nofuse
# nops inserted immediately before it in the ordered stream.
_PATCHED = False


def _apply_tile_patch():
    global _PATCHED
    if _PATCHED:
        return
    from concourse.tile import TileContext as TC
    from concourse.vector_clock import ScopedClock, VectorClock

    def _drain_and_barrier_split(self, tick_clock, wait_clock):
        gc = tick_clock.global_clock
        n = len(gc)
        for i in range(n):
            if gc[i] > 0:
                vec = [0] * n
                vec[i] = gc[i]
                ins = self.nc.sync.nop(nofuse=True, hint="tail_drain_wait")
                wait_clock.add_sem_waits(
                    ins.ins, ScopedClock({None: VectorClock(vec)})
                )
        self.nc.sync.drain()
        self.nc.all_engine_barrier()
        assert self.sems is not None
        popped = self.nc._tile_sem_poison_stack.pop()
        assert popped is self._sem_poison
        self.nc.clear_and_free_semaphores(list(self.sems.allocated().values()))
        self.nc.all_engine_barrier()

    TC._drain_and_barrier = _drain_and_barrier_split

    orig_lower = TC._lower_ordered_insts
    counter = [0]

    def _lower_split_waits(self, ordered):
        for bb_name, insts in ordered.items():
            new = []
            changed = False
            for inst in insts:
                si = inst.sync_info
                if si is not None and len(si.on_wait) > 1:
                    changed = True
                    waits = list(si.on_wait)
                    for w in waits[:-1]:
                        counter[0] += 1
                        new.append(
                            mybir.InstNoOp(
                                name=f"splitw-{counter[0]}",
                                sync_info=mybir.SyncInfo(
                                    on_wait=[w], on_update=[]
                                ),
                                bass_nofuse=True,
                                engine=inst.engine,
                            )
                        )
                    inst.sync_info = mybir.SyncInfo(
                        on_wait=[waits[-1]], on_update=list(si.on_update)
                    )
                new.append(inst)
            if changed:
                insts[:] = new
        return orig_lower(self, ordered)

    TC._lower_ordered_insts = _lower_split_waits
    _PATCHED = True


def _emit_body(nc, tc, pools, ext):
    """Emit one full attention computation (one sample)."""
    consts, big, epool, fin, ps_s_pool, ps_acc_pool = pools
    x_e, wqt_e, wkt_e, wvt_e, bq_e, bk_e, bv_e, gam_e, y_e = ext

    # ---- constants / weights ---------------------------------------------
    wqt_a = consts.tile([P, P], BF16, tag="wqt_a")
    wqt_b = consts.tile([P, P], BF16, tag="wqt_b")
    wkt_a = consts.tile([P, P], BF16, tag="wkt_a")
    wkt_b = consts.tile([P, P], BF16, tag="wkt_b")
    wvt_a = consts.tile([P, C], BF16, tag="wvt_a")
    wvt_b = consts.tile([P, C], BF16, tag="wvt_b")
    bq_t = consts.tile([P, 1], F32, tag="bq_t")
    bk_t = consts.tile([P, 1], F32, tag="bk_t")
    bv_t = consts.tile([P, C], F32, tag="bv_t")
    gam_t = consts.tile([P, 1], F32, tag="gam_t")
    ones = consts.tile([P, P], BF16, tag="ones")
    ones_f = consts.tile([P, P], F32, tag="ones_f")

    nc.sync.dma_start(out=wqt_a[:], in_=wqt_e[0:P, :])
    nc.sync.dma_start(out=wqt_b[:], in_=wqt_e[P : 2 * P, :])
    nc.sync.dma_start(out=wkt_a[:], in_=wkt_e[0:P, :])
    nc.sync.dma_start(out=wkt_b[:], in_=wkt_e[P : 2 * P, :])
    nc.sync.dma_start(out=wvt_a[:], in_=wvt_e[0:P, :])
    nc.sync.dma_start(out=wvt_b[:], in_=wvt_e[P : 2 * P, :])
    nc.sync.dma_start(out=bq_t[:], in_=bq_e[:])
    nc.sync.dma_start(out=bk_t[:], in_=bk_e[:])
    nc.sync.dma_start(out=bv_t[:], in_=bv_e[:])
    nc.sync.dma_start(out=gam_t[:], in_=gam_e[:])
    nc.vector.memset(ones[:], 1.0)
    nc.vector.memset(ones_f[:], 1.0)

    xf0 = big.tile([P, N], F32, tag="xf0")
    xf1 = big.tile([P, N], F32, tag="xf1")
    xb0 = big.tile([P, N], BF16, tag="xb0")
    xb1 = big.tile([P, N], BF16, tag="xb1")
    q_rep = big.tile([P, N], BF16, tag="q_rep")
    k_rep = big.tile([P, N], BF16, tag="k_rep")
    vt = big.tile([P, NJT * C], BF16, tag="vt")

    # ---- chunked x load + bf16 cast + q/k projections --------------------
    for nch in range(N // HCH):
        sl = slice(nch * HCH, (nch + 1) * HCH)
        nc.sync.dma_start(out=xf0[:, sl], in_=x_e[0:P, sl])
        nc.sync.dma_start(out=xf1[:, sl], in_=x_e[P : 2 * P, sl])
        nc.vector.tensor_copy(xb0[:, sl], xf0[:, sl])
        nc.vector.tensor_copy(xb1[:, sl], xf1[:, sl])
        pk = ps_acc_pool.tile([P, HCH], F32, tag="po", bufs=2)
        nc.tensor.matmul(pk[:], wkt_a[:], xb0[:, sl], start=True, stop=False)
        nc.tensor.matmul(pk[:], wkt_b[:], xb1[:, sl], start=False, stop=True)
        nc.vector.tensor_scalar_add(k_rep[:, sl], pk[:], bk_t[:])
    for nch in range(N // HCH):
        sl = slice(nch * HCH, (nch + 1) * HCH)
        pq = ps_acc_pool.tile([P, HCH], F32, tag="po", bufs=2)
        nc.tensor.matmul(pq[:], wqt_a[:], xb0[:, sl], start=True, stop=False)
        nc.tensor.matmul(pq[:], wqt_b[:], xb1[:, sl], start=False, stop=True)
        nc.vector.tensor_scalar_add(q_rep[:, sl], pq[:], bq_t[:])

    for jt in range(NJT):
        nsl = slice(jt * P, (jt + 1) * P)
        pv = ps_acc_pool.tile([P, C], F32, tag="po", bufs=2)
        nc.tensor.matmul(pv[:], xb0[:, nsl], wvt_a[:], start=True, stop=False)
        nc.tensor.matmul(pv[:], xb1[:, nsl], wvt_b[:], start=False, stop=True)
        nc.vector.tensor_tensor(
            vt[:, jt * C : (jt + 1) * C], pv[:], bv_t[:], op=ALU.add
        )


    # ---- attention main loop: i-chunks of 512 ----------------------------
    # Per chunk: scores'+exp for all 32 j-tiles (ACT-bound, PE lightly used),
    # then the accumulation matmuls chase the exp stream (PE-bound).
    NCH = N // HCH  # 8
    NJG = NJT // 4  # 8 j-groups of 4 j-tiles

    for ich in range(NCH):
        isl = slice(ich * HCH, (ich + 1) * HCH)
        ebigs = []
        for jg in range(NJG):
            ps_big = ps_s_pool.tile([P, 4 * HCH], F32, tag="ps_s")
            for g in range(4):
                jt = 4 * jg + g
                if VARIANT["s_tiled"]:
                    nc.tensor.matmul(
                        ps_big[:, g * HCH : (g + 1) * HCH],
                        k_rep[32 * g : 32 * (g + 1), jt * P : (jt + 1) * P],
                        q_rep[32 * g : 32 * (g + 1), isl],
                        start=True, stop=True,
                        tile_position=(32 * g, 0),
                    )
                else:
                    nc.tensor.matmul(
                        ps_big[:, g * HCH : (g + 1) * HCH],
                        k_rep[0:DK, jt * P : (jt + 1) * P],
                        q_rep[0:DK, isl],
                        start=True, stop=True,
                    )
            e_big = epool.tile([P, 4 * HCH], BF16, tag="e")
            nc.scalar.activation(e_big[:], ps_big[:], AF.Exp)
            ebigs.append(e_big)

        def esl(jt):
            return ebigs[jt // 4][:, (jt % 4) * HCH : (jt % 4 + 1) * HCH]

        po0 = ps_acc_pool.tile([P, HCH], F32, tag="po", bufs=2)
        po1 = ps_acc_pool.tile([P, HCH], F32, tag="po", bufs=2)
        for jt in range(NJT):
            st = jt == 0
            sp = jt == NJT - 1
            nc.tensor.matmul(
                po0[:], vt[:, jt * C : jt * C + P], esl(jt), start=st, stop=sp
            )
            nc.tensor.matmul(
                po1[:], vt[:, jt * C + P : (jt + 1) * C], esl(jt),
                start=st, stop=sp,
            )

        pd = ps_acc_pool.tile([P, HCH], F32, tag="pd", bufs=2)
        for jt in range(NJT):
            g = jt % 4
            if VARIANT["d_tiled"]:
                nc.tensor.matmul(
                    pd[32 * g : 32 * (g + 1), :], ones[:, 0:32], esl(jt),
                    start=(jt < 4), stop=(jt >= NJT - 4),
                    tile_position=(0, 32 * g),
                )
            else:
                nc.tensor.matmul(
                    pd[32 * g : 32 * (g + 1), :], ones[:, 0:32], esl(jt),
                    start=(jt < 4), stop=(jt >= NJT - 4),
                )

        # quadrant-sum (32x overcount folded into gamma), then finalize
        d_sb = fin.tile([P, HCH], mybir.dt.float32r, tag="d_sb")
        nc.vector.tensor_copy(d_sb[:], pd[:])
        nc.tensor.matmul(
            pd[:], ones_f[:].bitcast(mybir.dt.float32r), d_sb[:],
            start=True, stop=True,
        )
        dr = fin.tile([P, HCH], F32, tag="dr")
        nc.vector.reciprocal(dr[:], pd[:])
        nc.vector.tensor_scalar_mul(dr[:], dr[:], gam_t[:])
        t0 = fin.tile([P, HCH], F32, tag="t0")
        nc.vector.tensor_tensor(t0[:], po0[:], dr[:], op=ALU.mult)
        nc.vector.tensor_tensor(t0[:], t0[:], xf0[:, isl], op=ALU.add)
        nc.sync.dma_start(out=y_e[0:P, isl], in_=t0[:])
        t1 = fin.tile([P, HCH], F32, tag="t1")
        nc.vector.tensor_tensor(t1[:], po1[:], dr[:], op=ALU.mult)
        nc.vector.tensor_tensor(t1[:], t1[:], xf1[:, isl], op=ALU.add)
        nc.sync.dma_start(out=y_e[P : 2 * P, isl], in_=t1[:])

    # vT projection is emitted before this loop (see above)

def build_bass(loop_n: int | None = None) -> bass.Bass:
    """Build the kernel. loop_n wraps the body in a device-side For_i loop
    (with a tiny 'tick' sentinel output) for slope-based benchmarking."""
    _apply_tile_patch()
    nc = bass.Bass()

    x_e = nc.declare_dram_parameter("x", [C, N], F32, isOutput=False)
    wqt_e = nc.declare_dram_parameter("wqt", [C, P], BF16, isOutput=False)
    wkt_e = nc.declare_dram_parameter("wkt", [C, P], BF16, isOutput=False)
    wvt_e = nc.declare_dram_parameter("wvt", [C, C], BF16, isOutput=False)
    bq_e = nc.declare_dram_parameter("bq_r", [P, 1], F32, isOutput=False)
    bk_e = nc.declare_dram_parameter("bk_r", [P, 1], F32, isOutput=False)
    bv_e = nc.declare_dram_parameter("bv_b", [P, C], F32, isOutput=False)
    gam_e = nc.declare_dram_parameter("gam_b", [P, 1], F32, isOutput=False)
    y_e = nc.declare_dram_parameter("y", [C, N], F32, isOutput=True)
    tick_e = None
    if loop_n is not None:
        tick_e = nc.declare_dram_parameter("tick", [1, 8], F32, isOutput=True)

    ext = (x_e, wqt_e, wkt_e, wvt_e, bq_e, bk_e, bv_e, gam_e, y_e)

    with (
        TileContext(nc) as tc,
        tc.tile_pool(name="consts", bufs=1) as consts,
        tc.tile_pool(name="big", bufs=1) as big,
        tc.tile_pool(name="epool", bufs=18) as epool,
        tc.tile_pool(name="fin", bufs=2) as fin,
        tc.tile_pool(name="ps_s", bufs=1, space="PSUM") as ps_s_pool,
        tc.tile_pool(name="ps_acc", bufs=3, space="PSUM") as ps_acc_pool,
    ):
        pools = (consts, big, epool, fin, ps_s_pool, ps_acc_pool)
        if loop_n is None:
            _emit_body(nc, tc, pools, ext)
        else:
            with tc.For_i(0, loop_n, 1):
                _emit_body(nc, tc, pools, ext)
            t = fin.tile([1, 8], F32, tag="tick")
            nc.vector.memset(t[:], 1.0)
            nc.sync.dma_start(out=tick_e[:], in_=t[:])

    return nc


_NC_CACHE = None


def _get_nc() -> bass.Bass:
    global _NC_CACHE
    if _NC_CACHE is None:
        _NC_CACHE = build_bass()
    return _NC_CACHE


def prep_core_inputs(x, Wq, bq, Wk, bk, Wv, bv, gamma):
    x = np.asarray(x, np.float32).reshape(B, C, N)
    wqt = np.ascontiguousarray(np.tile(np.asarray(Wq, np.float32).T, (1, 4))).astype(ml_dtypes.bfloat16)
    wkt = np.ascontiguousarray(np.tile(np.asarray(Wk, np.float32).T, (1, 4))).astype(ml_dtypes.bfloat16)
    wvt = np.ascontiguousarray(np.asarray(Wv, np.float32).T).astype(ml_dtypes.bfloat16)
    bq_r = np.ascontiguousarray(np.tile(np.asarray(bq, np.float32), 4)).reshape(P, 1)
    bk_r = np.ascontiguousarray(np.tile(np.asarray(bk, np.float32), 4)).reshape(P, 1)
    bv_b = np.ascontiguousarray(np.broadcast_to(np.asarray(bv, np.float32), (P, C)))
    # NOTE: quadrant strip-sum replicates each quadrant sum over 32 rows, so
    # the all-ones reduction yields 32x the true denominator; compensate here.
    gam_b = np.full((P, 1), 32.0 * float(np.asarray(gamma).reshape(-1)[0]), np.float32)
    shared = {
        "wqt": wqt, "wkt": wkt, "wvt": wvt,
        "bq_r": bq_r, "bk_r": bk_r, "bv_b": bv_b, "gam_b": gam_b,
    }
    return [{"x": np.ascontiguousarray(x[b]), **shared} for b in range(B)]


def kernel(**inputs) -> np.ndarray:
    nc = _get_nc()
    in_maps = prep_core_inputs(**inputs)
    res = run_bass_kernel_spmd(nc, in_maps, list(range(B)))
    y = np.stack([res.results[i]["y"] for i in range(B)])
    return np.ascontiguousarray(y.reshape(B, C, H, W).astype(np.float32))


# revision 23
# speedup vs baseline: 1.2373x; 1.0609x over previous
"""Trainium2 Bass kernel for nn_Attention_9594956939856.

Single-head spatial self-attention over 64x64 feature maps:
    q = Wq@x, k = Wk@x, v = Wv@x  (1x1 convs over channels)
    out = gamma * softmax(q^T k) @ v + x

Sharding: data-parallel over batch — 8 samples onto 8 NeuronCores, each core
computes one full sample (C=256, N=4096 tokens, dk=32). No collectives.

Per-core layout strategy (matmuls on TensorE compute out = lhsT.T @ rhs):
  - scores are computed directly TRANSPOSED: s'[j,i] = sum_d k[d,j] q[d,i]
    with k j-tiles stationary, so the huge attention matrix never needs a
    transpose. q/k are replicated 4x along partitions (via host-replicated
    W^T) so the K=32 contraction can later use 4x row-tiled matmuls.
  - softmax denominator: ones(128,128) stationary sums exp(s') over
    partitions (j), accumulated across j-tiles in PSUM; M=128 broadcasts the
    sum to every output partition for free.
  - v is produced directly in transposed layout vT[n,e] by the projection
    (lhsT = x chunks, rhs = Wv^T) — exactly the stationary layout the
    attention-weighted sum needs.
  - exp on ScalarE in (128,1024) chunks (bf16 out), fp32 PSUM accumulation.
    Scores are in [-5,5] for this input distribution, so softmax without
    max-subtraction is numerically safe.
"""

import ml_dtypes
import numpy as np

import concourse.bass as bass
import concourse.mybir as mybir
from concourse.tile import TileContext
from concourse.bass_utils import run_bass_kernel_spmd

B, C, H, W = 8, 256, 64, 64
N = H * W          # 4096 tokens
DK = C // 8        # 32
P = 128
F32 = mybir.dt.float32
F32R = mybir.dt.float32r  # fp32 storage, single-pass (4x faster) PE streaming
BF16 = mybir.dt.bfloat16
FP8 = mybir.dt.float8e4
DR = mybir.MatmulPerfMode.DoubleRow
AF = mybir.ActivationFunctionType
ALU = mybir.AluOpType

NJT = N // P       # 32 j-tiles
ICH = 1024         # i-chunk width for the scores'/exp stage
NICH = N // ICH    # 4
HCH = 512          # accumulation sub-chunk (one PSUM bank)

# A/B flags (module-level so experiments can flip them before build)
VARIANT = {"s_tiled": True, "d_tiled": True}


# ---------------------------------------------------------------------------
# Workaround: the walrus build in this container allows only ONE sync wait
# per instruction ("Too many sync wait commands"), but Tile's wait
# assignment attaches up to 2 (and the tail drain more). Hoist all-but-one
# wait of any over-subscribed instruction onto dedicated same-engine nofuse
# nops inserted immediately before it in the ordered stream.
_PATCHED = False


def _apply_tile_patch():
    global _PATCHED
    if _PATCHED:
        return
    from concourse.tile import TileContext as TC
    from concourse.vector_clock import ScopedClock, VectorClock

    def _drain_and_barrier_split(self, tick_clock, wait_clock):
        gc = tick_clock.global_clock
        n = len(gc)
        for i in range(n):
            if gc[i] > 0:
                vec = [0] * n
                vec[i] = gc[i]
                ins = self.nc.sync.nop(nofuse=True, hint="tail_drain_wait")
                wait_clock.add_sem_waits(
                    ins.ins, ScopedClock({None: VectorClock(vec)})
                )
        self.nc.sync.drain()
        self.nc.all_engine_barrier()
        assert self.sems is not None
        popped = self.nc._tile_sem_poison_stack.pop()
        assert popped is self._sem_poison
        self.nc.clear_and_free_semaphores(list(self.sems.allocated().values()))
        self.nc.all_engine_barrier()

    TC._drain_and_barrier = _drain_and_barrier_split

    orig_lower = TC._lower_ordered_insts
    counter = [0]

    def _lower_split_waits(self, ordered):
        for bb_name, insts in ordered.items():
            new = []
            changed = False
            for inst in insts:
                si = inst.sync_info
                if si is not None and len(si.on_wait) > 1:
                    changed = True
                    waits = list(si.on_wait)
                    for w in waits[:-1]:
                        counter[0] += 1
                        new.append(
                            mybir.InstNoOp(
                                name=f"splitw-{counter[0]}",
                                sync_info=mybir.SyncInfo(
                                    on_wait=[w], on_update=[]
                                ),
                                bass_nofuse=True,
                                engine=inst.engine,
                            )
                        )
                    inst.sync_info = mybir.SyncInfo(
                        on_wait=[waits[-1]], on_update=list(si.on_update)
                    )
                new.append(inst)
            if changed:
                insts[:] = new
        return orig_lower(self, ordered)

    TC._lower_ordered_insts = _lower_split_waits
    _PATCHED = True


def _emit_body(nc, tc, pools, ext):
    """Emit one full attention computation (one sample)."""
    consts, big, epool, fin, ps_s_pool, ps_acc_pool = pools
    x_e, wqt_e, wkt_e, wvt_e, bq_e, bk_e, bv_e, gam_e, y_e = ext

    # ---- constants / weights ---------------------------------------------
    wqt_a = consts.tile([P, P], BF16, tag="wqt_a")
    wqt_b = consts.tile([P, P], BF16, tag="wqt_b")
    wkt_a = consts.tile([P, P], BF16, tag="wkt_a")
    wkt_b = consts.tile([P, P], BF16, tag="wkt_b")
    wvt_a = consts.tile([P, C], BF16, tag="wvt_a")
    wvt_b = consts.tile([P, C], BF16, tag="wvt_b")
    bq_t = consts.tile([P, 1], F32, tag="bq_t")
    bk_t = consts.tile([P, 1], F32, tag="bk_t")
    bv_t = consts.tile([P, C], F32, tag="bv_t")
    gam_t = consts.tile([P, 1], F32, tag="gam_t")
    ones = consts.tile([P, P], BF16, tag="ones")
    ones_f = consts.tile([P, P], F32, tag="ones_f")
    ones8 = consts.tile([P, 64], FP8, tag="ones8")

    nc.sync.dma_start(out=wqt_a[:], in_=wqt_e[0:P, :])
    nc.sync.dma_start(out=wqt_b[:], in_=wqt_e[P : 2 * P, :])
    nc.sync.dma_start(out=wkt_a[:], in_=wkt_e[0:P, :])
    nc.sync.dma_start(out=wkt_b[:], in_=wkt_e[P : 2 * P, :])
    nc.sync.dma_start(out=wvt_a[:], in_=wvt_e[0:P, :])
    nc.sync.dma_start(out=wvt_b[:], in_=wvt_e[P : 2 * P, :])
    nc.sync.dma_start(out=bq_t[:], in_=bq_e[:])
    nc.sync.dma_start(out=bk_t[:], in_=bk_e[:])
    nc.sync.dma_start(out=bv_t[:], in_=bv_e[:])
    nc.sync.dma_start(out=gam_t[:], in_=gam_e[:])
    nc.vector.memset(ones[:], 1.0)
    nc.vector.memset(ones_f[:], 1.0)
    nc.vector.memset(ones8[:], 1.0)

    xf0 = big.tile([P, N], F32, tag="xf0")
    xf1 = big.tile([P, N], F32, tag="xf1")
    xb0 = big.tile([P, N], BF16, tag="xb0")
    xb1 = big.tile([P, N], BF16, tag="xb1")
    q_rep = big.tile([P, N], BF16, tag="q_rep")
    k_rep = big.tile([P, N], BF16, tag="k_rep")
    # vt8: fp8 pair layout for DoubleRow mains — per j-pair jp (2 j-tiles)
    # and channel half h: cols [jp*512+h*256 : +256] = [vT(2jp) | vT(2jp+1)]
    vt8 = big.tile([P, (NJT // 2) * 512], FP8, tag="vt8")

    # ---- chunked x load + bf16 cast + q/k projections --------------------
    for nch in range(N // HCH):
        sl = slice(nch * HCH, (nch + 1) * HCH)
        nc.sync.dma_start(out=xf0[:, sl], in_=x_e[0:P, sl])
        nc.sync.dma_start(out=xf1[:, sl], in_=x_e[P : 2 * P, sl])
        nc.vector.tensor_copy(xb0[:, sl], xf0[:, sl])
        nc.vector.tensor_copy(xb1[:, sl], xf1[:, sl])
        pk = ps_acc_pool.tile([P, HCH], F32, tag="po", bufs=2)
        nc.tensor.matmul(pk[:], wkt_a[:], xb0[:, sl], start=True, stop=False)
        nc.tensor.matmul(pk[:], wkt_b[:], xb1[:, sl], start=False, stop=True)
        nc.vector.tensor_scalar_add(k_rep[:, sl], pk[:], bk_t[:])
    for nch in range(N // HCH):
        sl = slice(nch * HCH, (nch + 1) * HCH)
        pq = ps_acc_pool.tile([P, HCH], F32, tag="po", bufs=2)
        nc.tensor.matmul(pq[:], wqt_a[:], xb0[:, sl], start=True, stop=False)
        nc.tensor.matmul(pq[:], wqt_b[:], xb1[:, sl], start=False, stop=True)
        nc.vector.tensor_scalar_add(q_rep[:, sl], pq[:], bq_t[:])

    for jt in range(NJT):
        nsl = slice(jt * P, (jt + 1) * P)
        pv = ps_acc_pool.tile([P, C], F32, tag="po", bufs=2)
        nc.tensor.matmul(pv[:], xb0[:, nsl], wvt_a[:], start=True, stop=False)
        nc.tensor.matmul(pv[:], xb1[:, nsl], wvt_b[:], start=False, stop=True)
        jp, o = jt // 2, jt % 2
        base = jp * 512
        nc.vector.tensor_tensor(
            vt8[:, base + o * P : base + o * P + P],
            pv[:, 0:P], bv_t[:, 0:P], op=ALU.add,
        )
        nc.vector.tensor_tensor(
            vt8[:, base + 2 * P + o * P : base + 2 * P + o * P + P],
            pv[:, P:C], bv_t[:, P:C], op=ALU.add,
        )


    # ---- attention main loop: i-chunks of 512 ----------------------------
    # Per chunk: scores'+exp for all 32 j-tiles (ACT-bound, PE lightly used),
    # then the accumulation matmuls chase the exp stream (PE-bound).
    NCH = N // HCH  # 8
    NJG = NJT // 4  # 8 j-groups of 4 j-tiles

    for ich in range(NCH):
        isl = slice(ich * HCH, (ich + 1) * HCH)
        ebigs = []
        for jg in range(NJG):
            ps_big = ps_s_pool.tile([P, 4 * HCH], F32, tag="ps_s")
            for g in range(4):
                jt = 4 * jg + g
                if VARIANT["s_tiled"]:
                    nc.tensor.matmul(
                        ps_big[:, g * HCH : (g + 1) * HCH],
                        k_rep[32 * g : 32 * (g + 1), jt * P : (jt + 1) * P],
                        q_rep[32 * g : 32 * (g + 1), isl],
                        start=True, stop=True,
                        tile_position=(32 * g, 0),
                    )
                else:
                    nc.tensor.matmul(
                        ps_big[:, g * HCH : (g + 1) * HCH],
                        k_rep[0:DK, jt * P : (jt + 1) * P],
                        q_rep[0:DK, isl],
                        start=True, stop=True,
                    )
            # e8 pair layout (16-byte block interleave, as DoubleRow wants):
            # e8[p, a*1024 + (i//16)*32 + 16*o + i%16] = exp(s')[4jg+2a+o][p, i]
            e8 = epool.tile([P, 4 * HCH], FP8, tag="e")
            for a in range(2):
                asl = slice(a * 2 * HCH, (a + 1) * 2 * HCH)
                in_v = ps_big[:, asl].rearrange(
                    "p (o b r) -> p o b r", o=2, b=HCH // 16, r=16
                )
                out_v = e8[:, asl].rearrange(
                    "p (b o r) -> p o b r", b=HCH // 16, o=2, r=16
                )
                nc.scalar.activation(out_v, in_v, AF.Exp)
            ebigs.append(e8)

        po0 = ps_acc_pool.tile([P, HCH], F32, tag="po", bufs=2)
        po1 = ps_acc_pool.tile([P, HCH], F32, tag="po", bufs=2)
        pd = ps_acc_pool.tile([P, HCH], F32, tag="pd", bufs=2)
        NJP = NJT // 2
        for jp in range(NJP):
            rhs = ebigs[jp // 2][
                :, (jp % 2) * 2 * HCH : (jp % 2 + 1) * 2 * HCH
            ].rearrange("p (b o r) -> p o b r", b=HCH // 16, o=2, r=16)
            st = jp == 0
            sp = jp == NJP - 1
            for h, po in ((0, po0), (1, po1)):
                lhsT = vt8[
                    :, jp * 512 + h * 2 * P : jp * 512 + (h + 1) * 2 * P
                ].rearrange("p (o m) -> p o m", o=2)
                nc.tensor.matmul(
                    po[:], lhsT, rhs, start=st, stop=sp, perf_mode=DR
                )
            nc.tensor.matmul(
                pd[0:32, :],
                ones8[:].rearrange("p (o m) -> p o m", o=2),
                rhs, start=st, stop=sp, perf_mode=DR,
            )

        # quadrant-sum (32x overcount folded into gamma), then finalize
        d_sb = fin.tile([P, HCH], mybir.dt.float32r, tag="d_sb")
        nc.vector.tensor_copy(d_sb[0:32, :], pd[0:32, :])
        nc.tensor.matmul(
            pd[:], ones_f[0:32, :].bitcast(mybir.dt.float32r), d_sb[0:32, :],
            start=True, stop=True,
        )
        dr = fin.tile([P, HCH], F32, tag="dr")
        nc.vector.reciprocal(dr[:], pd[:])
        nc.vector.tensor_scalar_mul(dr[:], dr[:], gam_t[:])
        t0 = fin.tile([P, HCH], F32, tag="t0")
        nc.vector.tensor_tensor(t0[:], po0[:], dr[:], op=ALU.mult)
        nc.vector.tensor_tensor(t0[:], t0[:], xf0[:, isl], op=ALU.add)
        nc.sync.dma_start(out=y_e[0:P, isl], in_=t0[:])
        t1 = fin.tile([P, HCH], F32, tag="t1")
        nc.vector.tensor_tensor(t1[:], po1[:], dr[:], op=ALU.mult)
        nc.vector.tensor_tensor(t1[:], t1[:], xf1[:, isl], op=ALU.add)
        nc.sync.dma_start(out=y_e[P : 2 * P, isl], in_=t1[:])

    # vT projection is emitted before this loop (see above)

def build_bass(loop_n: int | None = None) -> bass.Bass:
    """Build the kernel. loop_n wraps the body in a device-side For_i loop
    (with a tiny 'tick' sentinel output) for slope-based benchmarking."""
    _apply_tile_patch()
    nc = bass.Bass()

    x_e = nc.declare_dram_parameter("x", [C, N], F32, isOutput=False)
    wqt_e = nc.declare_dram_parameter("wqt", [C, P], BF16, isOutput=False)
    wkt_e = nc.declare_dram_parameter("wkt", [C, P], BF16, isOutput=False)
    wvt_e = nc.declare_dram_parameter("wvt", [C, C], BF16, isOutput=False)
    bq_e = nc.declare_dram_parameter("bq_r", [P, 1], F32, isOutput=False)
    bk_e = nc.declare_dram_parameter("bk_r", [P, 1], F32, isOutput=False)
    bv_e = nc.declare_dram_parameter("bv_b", [P, C], F32, isOutput=False)
    gam_e = nc.declare_dram_parameter("gam_b", [P, 1], F32, isOutput=False)
    y_e = nc.declare_dram_parameter("y", [C, N], F32, isOutput=True)
    tick_e = None
    if loop_n is not None:
        tick_e = nc.declare_dram_parameter("tick", [1, 8], F32, isOutput=True)

    ext = (x_e, wqt_e, wkt_e, wvt_e, bq_e, bk_e, bv_e, gam_e, y_e)

    with (
        TileContext(nc) as tc,
        tc.tile_pool(name="consts", bufs=1) as consts,
        tc.tile_pool(name="big", bufs=1) as big,
        tc.tile_pool(name="epool", bufs=18) as epool,
        tc.tile_pool(name="fin", bufs=2) as fin,
        tc.tile_pool(name="ps_s", bufs=1, space="PSUM") as ps_s_pool,
        tc.tile_pool(name="ps_acc", bufs=3, space="PSUM") as ps_acc_pool,
    ):
        pools = (consts, big, epool, fin, ps_s_pool, ps_acc_pool)
        if loop_n is None:
            _emit_body(nc, tc, pools, ext)
        else:
            with tc.For_i(0, loop_n, 1):
                _emit_body(nc, tc, pools, ext)
            t = fin.tile([1, 8], F32, tag="tick")
            nc.vector.memset(t[:], 1.0)
            nc.sync.dma_start(out=tick_e[:], in_=t[:])

    return nc


_NC_CACHE = None


def _get_nc() -> bass.Bass:
    global _NC_CACHE
    if _NC_CACHE is None:
        _NC_CACHE = build_bass()
    return _NC_CACHE


def prep_core_inputs(x, Wq, bq, Wk, bk, Wv, bv, gamma):
    x = np.asarray(x, np.float32).reshape(B, C, N)
    wqt = np.ascontiguousarray(np.tile(np.asarray(Wq, np.float32).T, (1, 4))).astype(ml_dtypes.bfloat16)
    wkt = np.ascontiguousarray(np.tile(np.asarray(Wk, np.float32).T, (1, 4))).astype(ml_dtypes.bfloat16)
    wvt = np.ascontiguousarray(np.asarray(Wv, np.float32).T).astype(ml_dtypes.bfloat16)
    bq_r = np.ascontiguousarray(np.tile(np.asarray(bq, np.float32), 4)).reshape(P, 1)
    bk_r = np.ascontiguousarray(np.tile(np.asarray(bk, np.float32), 4)).reshape(P, 1)
    bv_b = np.ascontiguousarray(np.broadcast_to(np.asarray(bv, np.float32), (P, C)))
    # NOTE: quadrant strip-sum replicates each quadrant sum over 32 rows, so
    # the all-ones reduction yields 32x the true denominator; compensate here.
    gam_b = np.full((P, 1), 32.0 * float(np.asarray(gamma).reshape(-1)[0]), np.float32)
    shared = {
        "wqt": wqt, "wkt": wkt, "wvt": wvt,
        "bq_r": bq_r, "bk_r": bk_r, "bv_b": bv_b, "gam_b": gam_b,
    }
    return [{"x": np.ascontiguousarray(x[b]), **shared} for b in range(B)]


def kernel(**inputs) -> np.ndarray:
    nc = _get_nc()
    in_maps = prep_core_inputs(**inputs)
    res = run_bass_kernel_spmd(nc, in_maps, list(range(B)))
    y = np.stack([res.results[i]["y"] for i in range(B)])
    return np.ascontiguousarray(y.reshape(B, C, H, W).astype(np.float32))


# revision 25
# speedup vs baseline: 1.2729x; 1.0288x over previous
"""Trainium2 Bass kernel for nn_Attention_9594956939856.

Single-head spatial self-attention over 64x64 feature maps:
    q = Wq@x, k = Wk@x, v = Wv@x  (1x1 convs over channels)
    out = gamma * softmax(q^T k) @ v + x

Sharding: data-parallel over batch — 8 samples onto 8 NeuronCores, each core
computes one full sample (C=256, N=4096 tokens, dk=32). No collectives.

Per-core layout strategy (matmuls on TensorE compute out = lhsT.T @ rhs):
  - scores are computed directly TRANSPOSED: s'[j,i] = sum_d k[d,j] q[d,i]
    with k j-tiles stationary, so the huge attention matrix never needs a
    transpose. q/k are replicated 4x along partitions (via host-replicated
    W^T) so the K=32 contraction can later use 4x row-tiled matmuls.
  - softmax denominator: ones(128,128) stationary sums exp(s') over
    partitions (j), accumulated across j-tiles in PSUM; M=128 broadcasts the
    sum to every output partition for free.
  - v is produced directly in transposed layout vT[n,e] by the projection
    (lhsT = x chunks, rhs = Wv^T) — exactly the stationary layout the
    attention-weighted sum needs.
  - exp on ScalarE in (128,1024) chunks (bf16 out), fp32 PSUM accumulation.
    Scores are in [-5,5] for this input distribution, so softmax without
    max-subtraction is numerically safe.
"""

import ml_dtypes
import numpy as np

import concourse.bass as bass
import concourse.mybir as mybir
from concourse.tile import TileContext
from concourse.bass_utils import run_bass_kernel_spmd

B, C, H, W = 8, 256, 64, 64
N = H * W          # 4096 tokens
DK = C // 8        # 32
P = 128
F32 = mybir.dt.float32
F32R = mybir.dt.float32r  # fp32 storage, single-pass (4x faster) PE streaming
BF16 = mybir.dt.bfloat16
FP8 = mybir.dt.float8e4
DR = mybir.MatmulPerfMode.DoubleRow
AF = mybir.ActivationFunctionType
ALU = mybir.AluOpType

NJT = N // P       # 32 j-tiles
ICH = 1024         # i-chunk width for the scores'/exp stage
NICH = N // ICH    # 4
HCH = 512          # accumulation sub-chunk (one PSUM bank)

# A/B flags (module-level so experiments can flip them before build)
VARIANT = {"s_tiled": True, "d_tiled": True}


# ---------------------------------------------------------------------------
# Workaround: the walrus build in this container allows only ONE sync wait
# per instruction ("Too many sync wait commands"), but Tile's wait
# assignment attaches up to 2 (and the tail drain more). Hoist all-but-one
# wait of any over-subscribed instruction onto dedicated same-engine nofuse
# nops inserted immediately before it in the ordered stream.
_PATCHED = False


def _apply_tile_patch():
    global _PATCHED
    if _PATCHED:
        return
    from concourse.tile import TileContext as TC
    from concourse.vector_clock import ScopedClock, VectorClock

    def _drain_and_barrier_split(self, tick_clock, wait_clock):
        gc = tick_clock.global_clock
        n = len(gc)
        for i in range(n):
            if gc[i] > 0:
                vec = [0] * n
                vec[i] = gc[i]
                ins = self.nc.sync.nop(nofuse=True, hint="tail_drain_wait")
                wait_clock.add_sem_waits(
                    ins.ins, ScopedClock({None: VectorClock(vec)})
                )
        self.nc.sync.drain()
        self.nc.all_engine_barrier()
        assert self.sems is not None
        popped = self.nc._tile_sem_poison_stack.pop()
        assert popped is self._sem_poison
        self.nc.clear_and_free_semaphores(list(self.sems.allocated().values()))
        self.nc.all_engine_barrier()

    TC._drain_and_barrier = _drain_and_barrier_split

    orig_lower = TC._lower_ordered_insts
    counter = [0]

    def _lower_split_waits(self, ordered):
        for bb_name, insts in ordered.items():
            new = []
            changed = False
            for inst in insts:
                si = inst.sync_info
                if si is not None and len(si.on_wait) > 1:
                    changed = True
                    waits = list(si.on_wait)
                    for w in waits[:-1]:
                        counter[0] += 1
                        new.append(
                            mybir.InstNoOp(
                                name=f"splitw-{counter[0]}",
                                sync_info=mybir.SyncInfo(
                                    on_wait=[w], on_update=[]
                                ),
                                bass_nofuse=True,
                                engine=inst.engine,
                            )
                        )
                    inst.sync_info = mybir.SyncInfo(
                        on_wait=[waits[-1]], on_update=list(si.on_update)
                    )
                new.append(inst)
            if changed:
                insts[:] = new
        return orig_lower(self, ordered)

    TC._lower_ordered_insts = _lower_split_waits
    _PATCHED = True


def _emit_body(nc, tc, pools, ext):
    """Emit one full attention computation (one sample)."""
    consts, big, epool, fin, ps_s_pool, ps_acc_pool = pools
    x_e, wqt_e, wkt_e, wvt_e, bq_e, bk_e, bv_e, gam_e, y_e = ext

    # ---- constants / weights ---------------------------------------------
    wqt_a = consts.tile([P, P], BF16, tag="wqt_a")
    wqt_b = consts.tile([P, P], BF16, tag="wqt_b")
    wkt_a = consts.tile([P, P], BF16, tag="wkt_a")
    wkt_b = consts.tile([P, P], BF16, tag="wkt_b")
    wvt_a = consts.tile([P, C], BF16, tag="wvt_a")
    wvt_b = consts.tile([P, C], BF16, tag="wvt_b")
    bq_t = consts.tile([P, 1], F32, tag="bq_t")
    bk_t = consts.tile([P, 1], F32, tag="bk_t")
    bv_t = consts.tile([P, C], F32, tag="bv_t")
    gam_t = consts.tile([P, 1], F32, tag="gam_t")
    ones = consts.tile([P, P], BF16, tag="ones")
    ones_f = consts.tile([P, P], F32, tag="ones_f")
    ones8 = consts.tile([P, 64], FP8, tag="ones8")

    nc.sync.dma_start(out=wqt_a[:], in_=wqt_e[0:P, :])
    nc.sync.dma_start(out=wqt_b[:], in_=wqt_e[P : 2 * P, :])
    nc.sync.dma_start(out=wkt_a[:], in_=wkt_e[0:P, :])
    nc.sync.dma_start(out=wkt_b[:], in_=wkt_e[P : 2 * P, :])
    nc.sync.dma_start(out=wvt_a[:], in_=wvt_e[0:P, :])
    nc.sync.dma_start(out=wvt_b[:], in_=wvt_e[P : 2 * P, :])
    nc.sync.dma_start(out=bq_t[:], in_=bq_e[:])
    nc.sync.dma_start(out=bk_t[:], in_=bk_e[:])
    nc.sync.dma_start(out=bv_t[:], in_=bv_e[:])
    nc.sync.dma_start(out=gam_t[:], in_=gam_e[:])
    nc.vector.memset(ones[:], 1.0)
    nc.vector.memset(ones_f[:], 1.0)
    nc.vector.memset(ones8[:], 1.0)

    xf0 = big.tile([P, N], F32, tag="xf0")
    xf1 = big.tile([P, N], F32, tag="xf1")
    xb0 = big.tile([P, N], BF16, tag="xb0")
    xb1 = big.tile([P, N], BF16, tag="xb1")
    q_rep = big.tile([P, N], BF16, tag="q_rep")
    k_rep = big.tile([P, N], BF16, tag="k_rep")
    # vt8: fp8 pair layout for DoubleRow mains — per j-pair jp (2 j-tiles)
    # and channel half h: cols [jp*512+h*256 : +256] = [vT(2jp) | vT(2jp+1)]
    vt8 = big.tile([P, (NJT // 2) * 512], FP8, tag="vt8")

    # ---- chunked x load + bf16 cast + q/k projections --------------------
    for nch in range(N // HCH):
        sl = slice(nch * HCH, (nch + 1) * HCH)
        nc.sync.dma_start(out=xf0[:, sl], in_=x_e[0:P, sl])
        nc.sync.dma_start(out=xf1[:, sl], in_=x_e[P : 2 * P, sl])
        nc.vector.tensor_copy(xb0[:, sl], xf0[:, sl])
        nc.vector.tensor_copy(xb1[:, sl], xf1[:, sl])
        pk = ps_acc_pool.tile([P, HCH], F32, tag="po", bufs=2)
        nc.tensor.matmul(pk[:], wkt_a[:], xb0[:, sl], start=True, stop=False)
        nc.tensor.matmul(pk[:], wkt_b[:], xb1[:, sl], start=False, stop=True)
        nc.vector.tensor_scalar_add(k_rep[:, sl], pk[:], bk_t[:])
    for nch in range(N // HCH):
        sl = slice(nch * HCH, (nch + 1) * HCH)
        pq = ps_acc_pool.tile([P, HCH], F32, tag="po", bufs=2)
        nc.tensor.matmul(pq[:], wqt_a[:], xb0[:, sl], start=True, stop=False)
        nc.tensor.matmul(pq[:], wqt_b[:], xb1[:, sl], start=False, stop=True)
        nc.vector.tensor_scalar_add(q_rep[:, sl], pq[:], bq_t[:])

    for jt in range(NJT):
        nsl = slice(jt * P, (jt + 1) * P)
        pv = ps_acc_pool.tile([P, C], F32, tag="po", bufs=2)
        nc.tensor.matmul(pv[:], xb0[:, nsl], wvt_a[:], start=True, stop=False)
        nc.tensor.matmul(pv[:], xb1[:, nsl], wvt_b[:], start=False, stop=True)
        jp, o = jt // 2, jt % 2
        base = jp * 512
        nc.vector.tensor_tensor(
            vt8[:, base + o * P : base + o * P + P],
            pv[:, 0:P], bv_t[:, 0:P], op=ALU.add,
        )
        nc.vector.tensor_tensor(
            vt8[:, base + 2 * P + o * P : base + 2 * P + o * P + P],
            pv[:, P:C], bv_t[:, P:C], op=ALU.add,
        )


    # ---- attention main loop: i-chunks of 512 ----------------------------
    # Per chunk: scores'+exp for all 32 j-tiles (ACT-bound, PE lightly used),
    # then the accumulation matmuls chase the exp stream (PE-bound).
    NCH = N // HCH  # 8
    NJG = NJT // 4  # 8 j-groups of 4 j-tiles

    for ich in range(NCH):
        isl = slice(ich * HCH, (ich + 1) * HCH)
        ebigs = []
        for jg in range(NJG):
            ps_big = ps_s_pool.tile([P, 4 * HCH], F32, tag="ps_s")
            for g in range(4):
                jt = 4 * jg + g
                if VARIANT["s_tiled"]:
                    nc.tensor.matmul(
                        ps_big[:, g * HCH : (g + 1) * HCH],
                        k_rep[32 * g : 32 * (g + 1), jt * P : (jt + 1) * P],
                        q_rep[32 * g : 32 * (g + 1), isl],
                        start=True, stop=True,
                        tile_position=(32 * g, 0),
                    )
                else:
                    nc.tensor.matmul(
                        ps_big[:, g * HCH : (g + 1) * HCH],
                        k_rep[0:DK, jt * P : (jt + 1) * P],
                        q_rep[0:DK, isl],
                        start=True, stop=True,
                    )
            # e8 pair layout (16-byte block interleave, as DoubleRow wants):
            # e8[p, a*1024 + (i//16)*32 + 16*o + i%16] = exp(s')[4jg+2a+o][p, i]
            e8 = epool.tile([P, 4 * HCH], FP8, tag="e")
            for a in range(2):
                asl = slice(a * 2 * HCH, (a + 1) * 2 * HCH)
                in_v = ps_big[:, asl].rearrange(
                    "p (o b r) -> p o b r", o=2, b=HCH // 16, r=16
                )
                out_v = e8[:, asl].rearrange(
                    "p (b o r) -> p o b r", b=HCH // 16, o=2, r=16
                )
                nc.scalar.activation(out_v, in_v, AF.Exp)
            ebigs.append(e8)

        po0 = ps_acc_pool.tile([P, HCH], F32, tag="po", bufs=2)
        po1 = ps_acc_pool.tile([P, HCH], F32, tag="po", bufs=2)
        pd = ps_acc_pool.tile([P, HCH], F32, tag="pd", bufs=2)
        NJP = NJT // 2
        for jp in range(NJP):
            rhs = ebigs[jp // 2][
                :, (jp % 2) * 2 * HCH : (jp % 2 + 1) * 2 * HCH
            ].rearrange("p (b o r) -> p o b r", b=HCH // 16, o=2, r=16)
            st = jp == 0
            sp = jp == NJP - 1
            for h, po in ((0, po0), (1, po1)):
                lhsT = vt8[
                    :, jp * 512 + h * 2 * P : jp * 512 + (h + 1) * 2 * P
                ].rearrange("p (o m) -> p o m", o=2)
                nc.tensor.matmul(
                    po[:], lhsT, rhs, start=st, stop=sp, perf_mode=DR
                )
            nc.tensor.matmul(
                pd[0:32, :],
                ones8[:].rearrange("p (o m) -> p o m", o=2),
                rhs, start=st, stop=sp, perf_mode=DR,
            )

        # quadrant-sum (32x overcount folded into gamma), then finalize
        d_sb = fin.tile([P, HCH], mybir.dt.float32r, tag="d_sb")
        nc.vector.tensor_copy(d_sb[0:32, :], pd[0:32, :])
        nc.tensor.matmul(
            pd[:], ones_f[0:32, :].bitcast(mybir.dt.float32r), d_sb[0:32, :],
            start=True, stop=True,
        )
        dr = fin.tile([P, HCH], F32, tag="dr")
        nc.vector.reciprocal(dr[:], pd[:])
        nc.vector.tensor_scalar_mul(dr[:], dr[:], gam_t[:])
        t0 = fin.tile([P, HCH], F32, tag="t0")
        nc.vector.tensor_tensor(t0[:], po0[:], dr[:], op=ALU.mult)
        nc.vector.tensor_tensor(t0[:], t0[:], xf0[:, isl], op=ALU.add)
        nc.sync.dma_start(out=y_e[0:P, isl], in_=t0[:])
        t1 = fin.tile([P, HCH], F32, tag="t1")
        nc.vector.tensor_tensor(t1[:], po1[:], dr[:], op=ALU.mult)
        nc.vector.tensor_tensor(t1[:], t1[:], xf1[:, isl], op=ALU.add)
        nc.sync.dma_start(out=y_e[P : 2 * P, isl], in_=t1[:])

    # vT projection is emitted before this loop (see above)

def build_bass(loop_n: int | None = None) -> bass.Bass:
    """Build the kernel. loop_n wraps the body in a device-side For_i loop
    (with a tiny 'tick' sentinel output) for slope-based benchmarking."""
    _apply_tile_patch()
    nc = bass.Bass()

    x_e = nc.declare_dram_parameter("x", [C, N], F32, isOutput=False)
    wqt_e = nc.declare_dram_parameter("wqt", [C, P], BF16, isOutput=False)
    wkt_e = nc.declare_dram_parameter("wkt", [C, P], BF16, isOutput=False)
    wvt_e = nc.declare_dram_parameter("wvt", [C, C], BF16, isOutput=False)
    bq_e = nc.declare_dram_parameter("bq_r", [P, 1], F32, isOutput=False)
    bk_e = nc.declare_dram_parameter("bk_r", [P, 1], F32, isOutput=False)
    bv_e = nc.declare_dram_parameter("bv_b", [P, C], F32, isOutput=False)
    gam_e = nc.declare_dram_parameter("gam_b", [P, 1], F32, isOutput=False)
    y_e = nc.declare_dram_parameter("y", [C, N], F32, isOutput=True)
    tick_e = None
    if loop_n is not None:
        tick_e = nc.declare_dram_parameter("tick", [1, 8], F32, isOutput=True)

    ext = (x_e, wqt_e, wkt_e, wvt_e, bq_e, bk_e, bv_e, gam_e, y_e)

    with (
        TileContext(nc) as tc,
        tc.tile_pool(name="consts", bufs=1) as consts,
        tc.tile_pool(name="big", bufs=1) as big,
        tc.tile_pool(name="epool", bufs=18) as epool,
        tc.tile_pool(name="fin", bufs=2) as fin,
        tc.tile_pool(name="ps_s", bufs=1, space="PSUM") as ps_s_pool,
        tc.tile_pool(name="ps_acc", bufs=3, space="PSUM") as ps_acc_pool,
    ):
        pools = (consts, big, epool, fin, ps_s_pool, ps_acc_pool)
        if loop_n is None:
            _emit_body(nc, tc, pools, ext)
        else:
            with tc.For_i(0, loop_n, 1):
                _emit_body(nc, tc, pools, ext)
            t = fin.tile([1, 8], F32, tag="tick")
            nc.vector.memset(t[:], 1.0)
            nc.sync.dma_start(out=tick_e[:], in_=t[:])

    return nc


_NC_CACHE = None


def _get_nc() -> bass.Bass:
    global _NC_CACHE
    if _NC_CACHE is None:
        _NC_CACHE = build_bass()
    return _NC_CACHE


def prep_core_inputs(x, Wq, bq, Wk, bk, Wv, bv, gamma):
    x = np.asarray(x, np.float32).reshape(B, C, N)
    wqt = np.ascontiguousarray(np.tile(np.asarray(Wq, np.float32).T, (1, 4))).astype(ml_dtypes.bfloat16)
    wkt = np.ascontiguousarray(np.tile(np.asarray(Wk, np.float32).T, (1, 4))).astype(ml_dtypes.bfloat16)
    wvt = np.ascontiguousarray(np.asarray(Wv, np.float32).T).astype(ml_dtypes.bfloat16)
    bq_r = np.ascontiguousarray(np.tile(np.asarray(bq, np.float32), 4)).reshape(P, 1)
    bk_r = np.ascontiguousarray(np.tile(np.asarray(bk, np.float32), 4)).reshape(P, 1)
    bv_b = np.ascontiguousarray(np.broadcast_to(np.asarray(bv, np.float32), (P, C)))
    # NOTE: quadrant strip-sum replicates each quadrant sum over 32 rows, so
    # the all-ones reduction yields 32x the true denominator; compensate here.
    gam_b = np.full((P, 1), 32.0 * float(np.asarray(gamma).reshape(-1)[0]), np.float32)
    shared = {
        "wqt": wqt, "wkt": wkt, "wvt": wvt,
        "bq_r": bq_r, "bk_r": bk_r, "bv_b": bv_b, "gam_b": gam_b,
    }
    return [{"x": np.ascontiguousarray(x[b]), **shared} for b in range(B)]


def kernel(**inputs) -> np.ndarray:
    nc = _get_nc()
    in_maps = prep_core_inputs(**inputs)
    res = run_bass_kernel_spmd(nc, in_maps, list(range(B)))
    y = np.stack([res.results[i]["y"] for i in range(B)])
    return np.ascontiguousarray(y.reshape(B, C, H, W).astype(np.float32))
